# revision 48
# baseline (speedup 1.0000x reference)
"""MiMoV2 decoder layer (attention + noaux-tc MoE) on 8 Trainium2 cores.

Sharding: tensor-parallel attention (2 q heads + 1 kv head per core),
expert-parallel MoE (2 experts per core, dense over all 1024 tokens),
norms/gate replicated. Activations flow feature-major ("transposed",
[feature-partitions, token-free]) the whole way so matmuls chain without
activation transposes; per-token scales are applied via DMA-broadcast
row tiles. Residuals are folded into the collectives as h/8 per core.

Precision: fp32r for attention/norms, exact fp32 for the gate logits
(routing margins are ~5e-5 — keep that path bit-identical to the
reference flow), fp8 e4m3 + DoubleRow for the expert matmuls (weights
pre-scaled on host; descale folded into the rmsnorm row / combine
weights / residual). h-AllReduce in fp32; output ReduceScatter in fp16
with a scaled residual, unscaled in a tiny tail pass.

kernel(**inputs) takes the full unsharded inputs and returns the full
[1, 1024, 2048] output.
"""
import numpy as np
import ml_dtypes

import concourse.bass as bass
import concourse.tile as tile
from concourse import mybir, bacc
from concourse.bass_utils import run_bass_kernel_spmd

f32 = mybir.dt.float32
f32r = mybir.dt.float32r
f16 = mybir.dt.float16
bf16 = mybir.dt.bfloat16
f8 = mybir.dt.float8e4
AF = mybir.ActivationFunctionType
ALU = mybir.AluOpType
AX = mybir.AxisListType
DR = mybir.MatmulPerfMode.DoubleRow

H = 2048
NH = 16
NKV = 4
HD = 128
E = 16
DFF = 1024
T = 1024
EPS = 1e-6
THETA = 1000000.0
N_CORES = 8
RG = [list(range(N_CORES))]
NEG = -1e5  # causal-mask penalty; exp() underflows to exactly 0

# fp8 scales (powers of two; relative fp8 precision is scale-free, these
# just center the dynamic range safely below the TRN e4m3 max of 240)
SX = 4.0     # hidden states into gate_up
SG = 256.0   # gate_up weights
SA = 8.0     # activations into down proj (cw<=1 folded in after)
SD = 256.0   # down weights
SXG = SX * SG
SAD = SA * SD


def _build_nc(dbg_outputs=False):
    nc = bacc.Bacc("TRN2", target_bir_lowering=False, debug=False,
                   num_devices=N_CORES)

    def din(name, shape, dt=f32):
        return nc.dram_tensor(name, shape, dt, kind="ExternalInput").ap()

    hidden_t = din("hidden_t", [H, T])
    qkv_w_s = din("qkv_w_s", [H, 4 * HD])
    o_w_s = din("o_w_s", [2 * HD, H])
    gate_wt = din("gate_wt", [H, E])
    bias_in = din("bias_t", [128, E])
    sel_in = din("sel_t", [E, 2])
    w_gu = din("w_gu", [2, H, 2 * DFF], f8)
    w_dn = din("w_dn", [2, DFF, H], f8)
    cos_in = din("cosf", [128, T])
    sin_in = din("sinf", [128, T])
    mask_in = din("mask_t", [128, 128])
    eye_in = din("eye_t", [128, 128])
    reye_in = din("reye_t", [128, 128], f16)
    ones_in = din("ones_t", [128, 1])
    out_part = nc.dram_tensor("out_part", [256, T], f32,
                              kind="ExternalOutput").ap()
    dbg = None
    if dbg_outputs:
        dbg = {
            "h": nc.dram_tensor("dbg_h", [H, T], f32, kind="ExternalOutput").ap(),
            "lg": nc.dram_tensor("dbg_lg", [E, T], f32, kind="ExternalOutput").ap(),
        }

    with tile.TileContext(nc) as tc:
        _emit(nc, tc, hidden_t, qkv_w_s, o_w_s, gate_wt, bias_in, sel_in,
              w_gu, w_dn, cos_in, sin_in, mask_in, eye_in, reye_in,
              ones_in, out_part, dbg)
    nc.compile()
    return nc


def _emit(nc, tc, hidden_t, qkv_w_s, o_w_s, gate_wt, bias_in, sel_in,
          w_gu, w_dn, cos_in, sin_in, mask_in, eye_in, reye_in,
          ones_in, out_part, dbg=None):
    from contextlib import ExitStack

    def mm(out, lhsT, rhs, start, stop):
        nc.tensor.matmul(out, lhsT, rhs, start=start, stop=stop)

    def mm8(out, lhsT, rhs, start, stop):
        nc.tensor.matmul(out, lhsT, rhs, start=start, stop=stop,
                         perf_mode=DR)

    def tt(out, a, b, op):
        nc.vector.tensor_tensor(out=out, in0=a, in1=b, op=op)

    with ExitStack() as ctx:
        gconst = ctx.enter_context(tc.tile_pool(name="gconst", bufs=1))
        gdram = ctx.enter_context(tc.tile_pool(name="gdram", bufs=1,
                                               space="DRAM"))
        # expert-0 first-block gate_up weights, prefetched during attention:
        # issuing these DMAs while the h AllReduce is flying starves the
        # collective down to ~33GB/s
        w_pre = ctx.enter_context(tc.tile_pool(name="w_pre", bufs=16))

        eye = gconst.tile([128, 128], f32)
        mask = gconst.tile([128, 128], f32)
        ones_r = gconst.tile([128, 1], f32r)
        bias_sb = gconst.tile([128, E], f32)
        sel_sb = gconst.tile([E, 2], f32r)
        cos_sb = gconst.tile([128, T], f32)
        sin_sb = gconst.tile([128, T], f32)
        reye_sb = gconst.tile([128, 128], f16)
        eps1 = gconst.tile([1, 1], f32)
        nc.vector.memset(eps1[:], EPS)
        ones_f = gconst.tile([128, 1], f32)
        nc.vector.memset(ones_f[:], 1.0)
        nc.sync.dma_start(eye[:], eye_in[:])
        nc.sync.dma_start(reye_sb[:], reye_in[:])
        nc.sync.dma_start(mask[:], mask_in[:])
        nc.sync.dma_start(ones_r[:], ones_in[:].bitcast(f32r))
        nc.sync.dma_start(bias_sb[:], bias_in[:])
        nc.sync.dma_start(sel_sb[:], sel_in[:].bitcast(f32r))
        nc.sync.dma_start(cos_sb[:], cos_in[:])
        nc.sync.dma_start(sin_sb[:], sin_in[:])

        ar1_in_a = gdram.tile([H // 2, T], f16)
        ar1_in_b = gdram.tile([H // 2, T], f16)
        ar1_out_a = gdram.tile([H // 2, T], f16, addr_space="Shared")
        ar1_out_b = gdram.tile([H // 2, T], f16, addr_space="Shared")
        lg_in = gdram.tile([E, T], f32)
        lg_out = gdram.tile([E, T], f32, addr_space="Shared")
        ar2_q = [gdram.tile([512, T], f16, tag=f"ar2q{q}", name=f"ar2q{q}")
                 for q in range(4)]
        rs_q = [gdram.tile([64, T], f16, tag=f"rsq{q}", name=f"rsq{q}")
                for q in range(4)]
        warm_in = gdram.tile([128, 16], f32)
        warm_out = gdram.tile([128, 16], f32, addr_space="Shared")

        def h_src(k):
            return (ar1_out_a if k < 8 else ar1_out_b, (k % 8) * 128)
        rows = [gdram.tile([1, T], f32, tag=f"row{i}", name=f"row{i}")
                for i in range(7)]

        # tiny warm-up collective: absorbs first-collective setup cost
        # while attention runs; nothing depends on its output
        nc.sync.dma_start(warm_in[:], eye[:, 0:16])
        nc.gpsimd.collective_compute(
            "AllReduce", ALU.add, replica_groups=RG,
            ins=[warm_in.opt()], outs=[warm_out.opt()])

        # ================= Phase A: attention =================
        with ExitStack() as actx:
            a_keep = actx.enter_context(tc.tile_pool(name="a_keep", bufs=1))
            pa_row = actx.enter_context(tc.tile_pool(name="pa_row", bufs=1,
                                                     space="PSUM"))
            a_hid = actx.enter_context(tc.tile_pool(name="a_hid", bufs=1))
            a_w = actx.enter_context(tc.tile_pool(name="a_w", bufs=1))

            hid = a_hid.tile([128, 16, T], f32r)
            for k in range(16):
                nc.sync.dma_start(hid[:, k, :],
                                  hidden_t[128 * k:128 * k + 128, :].bitcast(f32r))
            wq = a_w.tile([128, 16, 512], f32r)
            for k in range(16):
                nc.sync.dma_start(wq[:, k, :],
                                  qkv_w_s[128 * k:128 * k + 128, :].bitcast(f32r))
            gw = a_w.tile([128, 16, E], f32)
            for k in range(16):
                nc.sync.dma_start(gw[:, k, :],
                                  gate_wt[128 * k:128 * k + 128, :])
            wpre_g, wpre_u = [], []
            for qq, lst in ((0, wpre_g), (2, wpre_u)):
                for kk in range(8):
                    wg = w_pre.tile([128, 2, 512], f8, tag="wpre",
                                    name="wpre")
                    for i in range(2):
                        r0 = 256 * kk + 128 * i
                        nc.sync.dma_start(
                            wg[:, i, :],
                            w_gu[0, r0:r0 + 128, 512 * qq:512 * qq + 512])
                    lst.append(wg)

            s_b = a_keep.tile([128, T], f32)
            cos_s = a_keep.tile([128, T], f32)
            sin_s = a_keep.tile([128, T], f32)
            qk = a_keep.tile([128, 3, T], f32r)
            vhat = a_keep.tile([128, T], f32r)
            v_tm = a_keep.tile([128, 8, 128], f32r)
            oT = a_keep.tile([128, 2, T], f32r)

            # --- rmsnorm scale s[t] = rsqrt(mean(x^2)+eps), broadcast ---
            ssum = pa_row.tile([1, T], f32, tag="row")
            with tc.tile_pool(name="a_sq", bufs=4) as a_sq:
                for k in range(16):
                    sq = a_sq.tile([128, T], f32r, tag="sq")
                    nc.vector.tensor_mul(sq[:], hid[:, k, :].bitcast(f32),
                                         hid[:, k, :].bitcast(f32))
                    for c in range(2):
                        mm(ssum[0:1, 512 * c:512 * c + 512], ones_r[:],
                           sq[:, 512 * c:512 * c + 512], k == 0, k == 15)
            srow = a_keep.tile([1, T], f32)
            tmp_row = a_keep.tile([1, T], f32)
            nc.scalar.activation(tmp_row[:], ssum[:], AF.Sqrt,
                                 bias=eps1[0:1, 0:1], scale=1.0 / H)
            nc.vector.reciprocal(srow[:], tmp_row[:])
            nc.sync.dma_start(rows[0][:], srow[:])
            nc.sync.dma_start(s_b[:], rows[0][:].partition_broadcast(128))
            nc.vector.tensor_mul(cos_s[:], cos_sb[:], s_b[:])
            nc.vector.tensor_mul(sin_s[:], sin_sb[:], s_b[:])

            # --- qkv projection (+rms-scale via cos_s/sin_s, + rope) ---
            with (
                tc.tile_pool(name="a_qps", bufs=2, space="PSUM") as a_qps,
                tc.tile_pool(name="a_tmp", bufs=2) as a_tmp,
            ):
                for ct in range(4):
                    qraw = a_tmp.tile([128, T], f32, tag="qraw")
                    for c in range(2):
                        qp = a_qps.tile([128, 512], f32, tag="qkvps")
                        for k in range(16):
                            mm(qp[:], wq[:, k, 128 * ct:128 * ct + 128],
                               hid[:, k, 512 * c:512 * c + 512], k == 0, k == 15)
                        if ct == 3:
                            nc.vector.tensor_mul(
                                vhat[:, 512 * c:512 * c + 512], qp[:],
                                s_b[:, 512 * c:512 * c + 512])
                        else:
                            nc.scalar.copy(qraw[:, 512 * c:512 * c + 512], qp[:])
                    if ct < 3:
                        xsw = a_tmp.tile([128, T], f32, tag="xsw")
                        nc.sync.dma_start(xsw[0:64, :], qraw[64:128, :])
                        nc.sync.dma_start(xsw[64:128, :], qraw[0:64, :])
                        t1 = a_tmp.tile([128, T], f32, tag="ropet1")
                        t2 = a_tmp.tile([128, T], f32, tag="ropet2")
                        nc.vector.tensor_mul(t1[:], qraw[:], cos_s[:])
                        nc.vector.tensor_mul(t2[:], xsw[:], sin_s[:])
                        nc.vector.tensor_add(qk[:, ct, :], t1[:], t2[:])

            # --- v to token-major via PE transpose ---
            with tc.tile_pool(name="a_pst", bufs=2, space="PSUM") as a_pst:
                for j in range(8):
                    tp = a_pst.tile([128, 128], f32, tag="vt")
                    nc.tensor.transpose(
                        tp[:], vhat[:, 128 * j:128 * j + 128].bitcast(f32),
                        eye[:])
                    nc.vector.tensor_copy(v_tm[:, j, :], tp[:])

            # --- attention per head: scoresT -> exp -> denom/av matmuls ---
            with (
                tc.tile_pool(name="a_E", bufs=4) as a_E,
                tc.tile_pool(name="a_psc", bufs=3, space="PSUM") as a_psc,
                tc.tile_pool(name="a_pso", bufs=1, space="PSUM") as a_pso,
                tc.tile_pool(name="a_db", bufs=2) as a_db,
            ):
                for h in range(2):
                    o_ps = a_pso.tile([128, T], f32, tag="ops")
                    den = pa_row.tile([1, T], f32, tag="row")
                    for j in range(8):
                        c0d = 128 * j
                        pieces = []
                        if c0d < 512:
                            pieces.append((c0d, 512, j == 0, j == 3))
                        pieces.append((max(c0d, 512), 1024, j == 0, j == 7))
                        Ej = a_E.tile([128, T], f32r, tag="E")
                        for (c0, c1, first, last) in pieces:
                            w = c1 - c0
                            sc = a_psc.tile([128, 512], f32, tag="sc")
                            mm(sc[:, :w], qk[:, 2, c0d:c0d + 128],
                               qk[:, h, c0:c1], True, True)
                            if c0 == c0d:
                                nc.vector.tensor_add(sc[:, 0:128],
                                                     sc[:, 0:128], mask[:])
                            nc.scalar.activation(Ej[:, c0 - c0d:c1 - c0d],
                                                 sc[:, :w], AF.Exp)
                        for (c0, c1, first, last) in pieces:
                            src = Ej[:, c0 - c0d:c1 - c0d]
                            mm(den[0:1, c0:c1], ones_r[:], src, first, last)
                            mm(o_ps[:, c0:c1], v_tm[:, j, :], src, first, last)
                    drow = a_db.tile([1, T], f32, tag="drow")
                    nc.vector.reciprocal(drow[:], den[:])
                    nc.sync.dma_start(rows[1 + h][:], drow[:])
                    db = a_db.tile([128, T], f32, tag="db")
                    nc.sync.dma_start(db[:],
                                      rows[1 + h][:].partition_broadcast(128))
                    for c in range(2):
                        nc.vector.tensor_mul(oT[:, h, 512 * c:512 * c + 512],
                                             o_ps[:, 512 * c:512 * c + 512],
                                             db[:, 512 * c:512 * c + 512])

            # --- o-proj partial + residual/8 -> ar1_in (fp16), plus the
            # exact fp32 partial gate logits off the fp32 st before the
            # fp16 rounding; their small AllReduce queues after the h ones ---
            with (
                tc.tile_pool(name="a_ow", bufs=1) as a_ow,
                tc.tile_pool(name="a_st", bufs=4) as a_st,
                tc.tile_pool(name="a_psp", bufs=2, space="PSUM") as a_psp,
                tc.tile_pool(name="a_lgp", bufs=1, space="PSUM") as a_lgp,
            ):
                ow = a_ow.tile([128, 2, H], f32r)
                for kc in range(2):
                    nc.sync.dma_start(ow[:, kc, :],
                                      o_w_s[128 * kc:128 * kc + 128, :].bitcast(f32r))
                lgp = [a_lgp.tile([E, 512], f32, tag=f"lgp{c}",
                                  name=f"lgp{c}") for c in range(2)]
                for ht in range(16):
                    dst = ar1_in_a if ht < 8 else ar1_in_b
                    r0 = (ht % 8) * 128
                    for c in range(2):
                        yp = a_psp.tile([128, 512], f32, tag="op")
                        for kc in range(2):
                            mm(yp[:], ow[:, kc, 128 * ht:128 * ht + 128],
                               oT[:, kc, 512 * c:512 * c + 512], kc == 0, kc == 1)
                        st = a_st.tile([128, 512], f32, tag="ar1st")
                        nc.vector.scalar_tensor_tensor(
                            out=st[:], in0=hid[:, ht, 512 * c:512 * c + 512].bitcast(f32),
                            scalar=1.0 / N_CORES, in1=yp[:],
                            op0=ALU.mult, op1=ALU.add)
                        st16 = a_st.tile([128, 512], f16, tag="st16")
                        nc.vector.tensor_copy(st16[:], st[:])
                        nc.sync.dma_start(
                            dst[r0:r0 + 128, 512 * c:512 * c + 512], st16[:])
                        mm(lgp[c][:], gw[:, ht, :], st[:], ht == 0, ht == 15)
                    if ht == 7:
                        nc.gpsimd.collective_compute(
                            "AllReduce", ALU.add, replica_groups=RG,
                            ins=[ar1_in_a.opt()], outs=[ar1_out_a.opt()])
                for c in range(2):
                    lgs = a_st.tile([E, 512], f32, tag="lgs")
                    nc.scalar.copy(lgs[:], lgp[c][:])
                    nc.sync.dma_start(lg_in[0:E, 512 * c:512 * c + 512],
                                      lgs[:])

        nc.gpsimd.collective_compute(
            "AllReduce", ALU.add, replica_groups=RG,
            ins=[ar1_in_b.opt()], outs=[ar1_out_b.opt()])
        nc.gpsimd.collective_compute(
            "AllReduce", ALU.add, replica_groups=RG,
            ins=[lg_in.opt()], outs=[lg_out.opt()])

        # ================= Phase B: MoE =================
        with ExitStack() as bctx:
            b_keep = bctx.enter_context(tc.tile_pool(name="b_keep", bufs=1))
            b_h = bctx.enter_context(tc.tile_pool(name="b_h", bufs=3))

            x2q = b_keep.tile([128, 16, T], f8)
            s2qb = b_keep.tile([128, T], f32)
            s2bE = b_keep.tile([E, T], f32)
            cwT = b_keep.tile([E, T], f32r)
            cwsa_b = [b_keep.tile([128, T], f32, tag=f"cwb{e}", name=f"cwb{e}")
                      for e in range(2)]
            act_sh = b_keep.tile([128, 8, T], bf16)  # shared by both experts
            act = [act_sh, act_sh]
            actq = [b_keep.tile([128, 8, T], f8, tag=f"actq{e}", name=f"actq{e}")
                    for e in range(2)]
            lg_sb = b_keep.tile([E, T], f32)
            # --- one pass over h: x2 quantize (fp8) + rms2 sums; the raw
            # gate logits arrive exact from the fp32 AllReduce. s2 applied
            # later so everything streams as soon as each AR chunk lands. ---
            with (
                tc.tile_pool(name="pb_row", bufs=1, space="PSUM") as pb_row,
                tc.tile_pool(name="b_sq", bufs=3) as b_sq,
            ):
                s2sum = pb_row.tile([1, T], f32, tag="s2")
                for k in range(16):
                    src, r0 = h_src(k)
                    hk = b_h.tile([128, T], f16, tag="hk")
                    nc.sync.dma_start(hk[:], src[r0:r0 + 128, :])
                    nc.scalar.activation(x2q[:, k, :], hk[:], AF.Copy,
                                         scale=SX)
                    sq = b_sq.tile([128, T], f32r, tag="sq2")
                    nc.vector.tensor_mul(sq[:], hk[:], hk[:])
                    for c in range(2):
                        mm(s2sum[0:1, 512 * c:512 * c + 512], ones_r[:],
                           sq[:, 512 * c:512 * c + 512], k == 0, k == 15)
                s2row = b_keep.tile([1, T], f32)
                t2row = b_keep.tile([1, T], f32)
                s2qrow = b_keep.tile([1, T], f32)
                nc.scalar.activation(t2row[:], s2sum[:], AF.Sqrt,
                                     bias=eps1[0:1, 0:1], scale=1.0 / H)
                nc.vector.reciprocal(s2row[:], t2row[:])
                nc.scalar.activation(s2qrow[:], s2row[:], AF.Copy,
                                     scale=1.0 / SXG)
                nc.sync.dma_start(rows[3][:], s2row[:])
                nc.sync.dma_start(rows[4][:], s2qrow[:])
                nc.sync.dma_start(s2bE[:], rows[3][:].partition_broadcast(E))
                nc.sync.dma_start(s2qb[:], rows[4][:].partition_broadcast(128))
                lgraw = b_keep.tile([E, T], f32)
                nc.sync.dma_start(lgraw[:], lg_out[:])
                nc.vector.tensor_mul(lg_sb[:], lgraw[:], s2bE[:])
                if dbg is not None:
                    nc.sync.dma_start(dbg["lg"][:], lg_sb[:])
                    nc.sync.dma_start(dbg["h"][0:H // 2, :], ar1_out_a[:])
                    nc.sync.dma_start(dbg["h"][H // 2:H, :], ar1_out_b[:])

            # --- routing + experts. Emission order matters: the PE stream is
            # [lgT transposes][gate_up e0][cwT transposes+sel][gate_up e1]
            # [down] so the PE never waits on the (vector) top-k math; the
            # vector routing chain is emitted after e0's first weight block
            # so gate_up's silu stream isn't queued behind it. ---
            with (
                tc.tile_pool(name="b_rt", bufs=1) as rt,
                tc.tile_pool(name="b_wgu", bufs=36) as b_wgu,
                tc.tile_pool(name="b_gups", bufs=4, space="PSUM") as b_gups,
                tc.tile_pool(name="b_et", bufs=3) as b_et,
            ):
                # routing tiles (all-j batched: [128 tokens, 8 tiles, ...])
                lt_all = rt.tile([128, 8, E], f32)
                with tc.tile_pool(name="b_ltp", bufs=2,
                                  space="PSUM") as b_ltp:
                    for j in range(8):
                        tpj = b_ltp.tile([128, E], f32, tag="ltp")
                        nc.tensor.transpose(tpj[:],
                                            lg_sb[:, 128 * j:128 * j + 128],
                                            eye[0:E, 0:E])
                        nc.vector.tensor_copy(lt_all[:, j, :], tpj[:])

                def emit_gate_up(e, qg, qu, pre=None):
                    if pre is not None:
                        wgt_g, wgt_u = pre
                    else:
                        wgt_g = []
                        wgt_u = []
                        for qq, lst in ((qg, wgt_g), (qu, wgt_u)):
                            for kk in range(8):
                                wg = b_wgu.tile([128, 2, 512], f8, tag="wgu")
                                for i in range(2):
                                    r0 = 256 * kk + 128 * i
                                    nc.sync.dma_start(
                                        wg[:, i, :],
                                        w_gu[e, r0:r0 + 128,
                                             512 * qq:512 * qq + 512])
                                lst.append(wg)
                    for fl in range(4):
                        pg = 4 * qg + fl
                        fs = slice(128 * fl, 128 * fl + 128)
                        for c in range(2):
                            cs = slice(512 * c, 512 * c + 512)
                            gps = b_gups.tile([128, 512], f32, tag="gu")
                            for kk in range(8):
                                mm8(gps[:], wgt_g[kk][:, :, fs],
                                    x2q[:, 2 * kk:2 * kk + 2, cs],
                                    kk == 0, kk == 7)
                            ups = b_gups.tile([128, 512], f32, tag="gu")
                            for kk in range(8):
                                mm8(ups[:], wgt_u[kk][:, :, fs],
                                    x2q[:, 2 * kk:2 * kk + 2, cs],
                                    kk == 0, kk == 7)
                            g2 = b_et.tile([128, 512], f32, tag="g2")
                            nc.vector.tensor_mul(g2[:], gps[:], s2qb[:, cs])
                            sil = b_et.tile([128, 512], f32, tag="sil")
                            nc.scalar.activation(sil[:], g2[:], AF.Silu)
                            tm = b_et.tile([128, 512], f32, tag="tm")
                            nc.vector.tensor_mul(tm[:], ups[:], s2qb[:, cs])
                            nc.vector.tensor_mul(act[e][:, pg, cs], tm[:],
                                                 sil[:])

                def emit_routing_vector():
                    # all-j batched top-2-group / top-4-expert selection
                    sig_all = rt.tile([128, 8, E], f32)
                    nc.scalar.activation(sig_all[:], lt_all[:], AF.Sigmoid)
                    sb_all = rt.tile([128, 8, E], f32)
                    bias_rep = bass.AP(
                        tensor=bias_sb.tensor, offset=bias_sb.offset,
                        ap=[list(bias_sb.ap[0]), [0, 8], list(bias_sb.ap[1])])
                    tt(sb_all[:], sig_all[:], bias_rep, ALU.add)
                    sb4 = sb_all[:].rearrange("p j (g e) -> p j g e", g=4, e=4)
                    ga = rt.tile([128, 8, 4], f32)
                    gb = rt.tile([128, 8, 4], f32)
                    gc_ = rt.tile([128, 8, 4], f32)
                    gd = rt.tile([128, 8, 4], f32)
                    tt(ga[:], sb4[:, :, :, 0], sb4[:, :, :, 1], ALU.max)
                    tt(gb[:], sb4[:, :, :, 0], sb4[:, :, :, 1], ALU.min)
                    tt(gc_[:], sb4[:, :, :, 2], sb4[:, :, :, 3], ALU.max)
                    tt(gd[:], sb4[:, :, :, 2], sb4[:, :, :, 3], ALU.min)
                    t1_ = rt.tile([128, 8, 4], f32)
                    m1 = rt.tile([128, 8, 4], f32)
                    m2 = rt.tile([128, 8, 4], f32)
                    t2_ = rt.tile([128, 8, 4], f32)
                    tt(t1_[:], ga[:], gc_[:], ALU.max)
                    tt(m1[:], ga[:], gc_[:], ALU.min)
                    tt(m2[:], gb[:], gd[:], ALU.max)
                    tt(t2_[:], m1[:], m2[:], ALU.max)
                    gs = rt.tile([128, 8, 4], f32)
                    nc.vector.tensor_add(gs[:], t1_[:], t2_[:])
                    # top-2 of the 4 group scores -> threshold per (tok, j)
                    a2 = rt.tile([128, 8], f32)
                    b2 = rt.tile([128, 8], f32)
                    c2 = rt.tile([128, 8], f32)
                    d2 = rt.tile([128, 8], f32)
                    tt(a2[:], gs[:, :, 0], gs[:, :, 1], ALU.max)
                    tt(b2[:], gs[:, :, 0], gs[:, :, 1], ALU.min)
                    tt(c2[:], gs[:, :, 2], gs[:, :, 3], ALU.max)
                    tt(d2[:], gs[:, :, 2], gs[:, :, 3], ALU.min)
                    e2 = rt.tile([128, 8], f32)
                    f2 = rt.tile([128, 8], f32)
                    thr = rt.tile([128, 8], f32)
                    tt(e2[:], a2[:], c2[:], ALU.min)
                    tt(f2[:], b2[:], d2[:], ALU.max)
                    tt(thr[:], e2[:], f2[:], ALU.max)
                    thr_rep = bass.AP(
                        tensor=thr.tensor, offset=thr.offset,
                        ap=[list(thr.ap[0]), [1, 8], [0, 4]])
                    gmask = rt.tile([128, 8, 4], f32)
                    tt(gmask[:], gs[:], thr_rep, ALU.is_ge)
                    pen = rt.tile([128, 8, 4], f32)
                    nc.scalar.activation(pen[:], gmask[:], AF.Copy,
                                         scale=-NEG, bias=NEG)
                    pen_rep = bass.AP(
                        tensor=pen.tensor, offset=pen.offset,
                        ap=[list(pen.ap[0]), [4, 8], [1, 4], [0, 4]])
                    masked = rt.tile([128, 8, E], f32)
                    m4 = masked[:].rearrange("p j (g e) -> p j g e", g=4, e=4)
                    nc.vector.tensor_tensor(out=m4, in0=sb4, in1=pen_rep,
                                            op=ALU.add)
                    top8 = rt.tile([128, 64], f32)
                    for j in range(8):
                        nc.vector.max(top8[:, 8 * j:8 * j + 8],
                                      masked[:, j, :])
                    kth_rep = bass.AP(
                        tensor=top8.tensor, offset=top8.offset + 3,
                        ap=[list(top8.ap[0]), [8, 8], [0, E]])
                    selm = rt.tile([128, 8, E], f32)
                    tt(selm[:], masked[:], kth_rep, ALU.is_ge)
                    wgt = rt.tile([128, 8, E], f32)
                    nc.vector.tensor_mul(wgt[:], selm[:], sig_all[:])
                    dsum = rt.tile([128, 8], f32)
                    nc.vector.tensor_reduce(out=dsum[:], in_=wgt[:],
                                            axis=AX.X, op=ALU.add)
                    nc.vector.tensor_scalar_add(dsum[:], dsum[:], 1e-20)
                    rec = rt.tile([128, 8], f32)
                    nc.vector.reciprocal(rec[:], dsum[:])
                    rec_rep = bass.AP(
                        tensor=rec.tensor, offset=rec.offset,
                        ap=[list(rec.ap[0]), [1, 8], [0, E]])
                    cwtok = rt.tile([128, 8, E], f32)
                    tt(cwtok[:], wgt[:], rec_rep, ALU.mult)
                    return cwtok

                def emit_cw_select(cwtok):
                    # transpose per j-tile, select this core's 2 expert rows,
                    # broadcast over partitions with the fp8 scale folded in
                    with tc.tile_pool(name="b_sel", bufs=1,
                                      space="PSUM") as b_sel:
                        for j in range(8):
                            ctp = b_sel.tile([E, 128], f32, tag="ctp",
                                             bufs=2)
                            nc.tensor.transpose(ctp[:], cwtok[:, j, :],
                                                eye[:])
                            nc.vector.tensor_copy(
                                cwT[:, 128 * j:128 * j + 128], ctp[:])
                        cwl_sb = rt.tile([2, T], f32)
                        for c in range(2):
                            cs = slice(512 * c, 512 * c + 512)
                            cwl_ps = b_sel.tile([2, 512], f32, tag="cwl")
                            mm(cwl_ps[:], sel_sb[:], cwT[:, cs], True, True)
                            nc.scalar.activation(cwl_sb[:, cs], cwl_ps[:],
                                                 AF.Copy, scale=SA)
                    for e in range(2):
                        nc.sync.dma_start(rows[5 + e][:], cwl_sb[e:e + 1, :])
                        nc.sync.dma_start(
                            cwsa_b[e][:],
                            rows[5 + e][:].partition_broadcast(128))

                def emit_act_quant(e):
                    for pg in range(8):
                        nc.vector.tensor_mul(actq[e][:, pg, :],
                                             act[e][:, pg, :], cwsa_b[e][:])

                emit_gate_up(0, 0, 2, pre=(wpre_g, wpre_u))
                cwtok = emit_routing_vector()
                emit_gate_up(0, 1, 3)
                emit_cw_select(cwtok)
                emit_act_quant(0)
                emit_gate_up(1, 0, 2)
                emit_gate_up(1, 1, 3)
                emit_act_quant(1)

            # --- down proj + scaled residual -> 4 chunked fp16 RS ---
            with (
                tc.tile_pool(name="b_wdn", bufs=18) as b_wdn,
                tc.tile_pool(name="b_yps", bufs=3, space="PSUM") as b_yps,
                tc.tile_pool(name="b_st", bufs=4) as b_st,
            ):
                for hq in range(4):
                    hs_ = slice(512 * hq, 512 * hq + 512)
                    wdt = []
                    for e in range(2):
                        for m in range(4):
                            wd = b_wdn.tile([128, 2, 512], f8, tag="wdn")
                            for i in range(2):
                                r0 = 256 * m + 128 * i
                                nc.sync.dma_start(
                                    wd[:, i, :], w_dn[e, r0:r0 + 128, hs_])
                            wdt.append((e, m, wd))
                    for hl in range(4):
                        ht = 4 * hq + hl
                        fs = slice(128 * hl, 128 * hl + 128)
                        src, r0 = h_src(ht)
                        hk = b_h.tile([128, T], f16, tag="hk2")
                        nc.sync.dma_start(hk[:], src[r0:r0 + 128, :])
                        for c in range(2):
                            cs = slice(512 * c, 512 * c + 512)
                            yp = b_yps.tile([128, 512], f32, tag="y")
                            for i, (e, m, wd) in enumerate(wdt):
                                mm8(yp[:], wd[:, :, fs],
                                    actq[e][:, 2 * m:2 * m + 2, cs],
                                    i == 0, False)
                            # scaled residual via an f16 identity matmul into
                            # the same PSUM chain; copy-out on the (idle)
                            # scalar engine keeps DVE off this path entirely
                            mm(yp[:], reye_sb[:], hk[:, cs], False, True)
                            st = b_st.tile([128, 512], f16, tag="ar2st")
                            nc.scalar.activation(st[:], yp[:], AF.Copy)
                            nc.sync.dma_start(
                                ar2_q[hq][128 * hl:128 * hl + 128, cs], st[:])
                    nc.gpsimd.collective_compute(
                        "ReduceScatter", ALU.add, replica_groups=RG,
                        ins=[ar2_q[hq].opt()], outs=[rs_q[hq].opt()])

            # --- tail: unscale the fp16 RS output back to fp32 ---
            with tc.tile_pool(name="b_out", bufs=4) as b_out:
                for q in range(4):
                    rsb = b_out.tile([64, T], f16, tag="rsb")
                    nc.sync.dma_start(rsb[:], rs_q[q][:])
                    osb = b_out.tile([64, T], f32, tag="osb")
                    nc.scalar.activation(osb[:], rsb[:], AF.Copy,
                                         scale=1.0 / SAD)
                    nc.sync.dma_start(out_part[64 * q:64 * q + 64, :], osb[:])


_NC_CACHE = {}


def _get_nc(dbg_outputs=False):
    key = ("dbg" if dbg_outputs else "nc")
    if key not in _NC_CACHE:
        _NC_CACHE[key] = _build_nc(dbg_outputs)
    return _NC_CACHE[key]


def _make_in_maps(inputs):
    hidden = np.asarray(inputs["hidden_states"], dtype=np.float32)
    hidden_t = np.ascontiguousarray(hidden.reshape(T, H).T)
    pos = np.asarray(inputs["positions"]).reshape(T).astype(np.float32)
    in_norm = np.asarray(inputs["in_norm_w"], dtype=np.float32)
    post_norm = np.asarray(inputs["post_norm_w"], dtype=np.float32)
    qkv_w = np.asarray(inputs["qkv_w"], dtype=np.float32)
    o_w = np.asarray(inputs["o_w"], dtype=np.float32)
    gate_w = np.asarray(inputs["gate_w"], dtype=np.float32)
    gate_bias = np.asarray(inputs["gate_bias"], dtype=np.float32)
    gate_up_w = np.asarray(inputs["gate_up_w"], dtype=np.float32)
    down_w = np.asarray(inputs["down_w"], dtype=np.float32)

    # rope tables (match fp32 reference numerics)
    half = HD // 2
    inv_freq = (1.0 / (THETA ** (np.arange(half, dtype=np.float32) / half))
                ).astype(np.float32)
    ang = inv_freq[:, None] * pos[None, :]  # [64, T]
    cos64 = np.cos(ang).astype(np.float32)
    sin64 = np.sin(ang).astype(np.float32)
    cosf = np.ascontiguousarray(np.concatenate([cos64, cos64], axis=0))
    sinf = np.ascontiguousarray(np.concatenate([-sin64, sin64], axis=0))

    ii = np.arange(128)
    mask_t = np.where(ii[None, :] >= ii[:, None], 0.0, NEG).astype(np.float32)
    eye_t = np.eye(128, dtype=np.float32)
    reye_t = (np.eye(128) * (SAD / N_CORES)).astype(np.float16)
    ones_t = np.ones((128, 1), np.float32)
    bias_t = np.ascontiguousarray(np.tile(gate_bias[None, :], (128, 1)))

    qkv_scaled = qkv_w * in_norm[:, None]
    qkv_scaled[:, :NH * HD] *= HD ** -0.5
    gate_wt = np.ascontiguousarray(post_norm[:, None] * gate_w.T)
    f8np = ml_dtypes.float8_e4m3
    gu_f = np.clip(gate_up_w * post_norm[None, :, None] * SG,
                   -240, 240).astype(f8np)
    dn_f = np.clip(down_w * SD, -240, 240).astype(f8np)

    in_maps = []
    for c in range(N_CORES):
        kvh = c // 2
        qc = qkv_scaled[:, 256 * c:256 * c + 256]
        kc = qkv_scaled[:, NH * HD + HD * kvh: NH * HD + HD * kvh + HD]
        vc = qkv_scaled[:, (NH + NKV) * HD + HD * kvh:
                        (NH + NKV) * HD + HD * kvh + HD]
        sel = np.zeros((E, 2), np.float32)
        sel[2 * c, 0] = 1.0
        sel[2 * c + 1, 1] = 1.0
        in_maps.append({
            "hidden_t": hidden_t,
            "qkv_w_s": np.ascontiguousarray(np.concatenate([qc, kc, vc], axis=1)),
            "o_w_s": np.ascontiguousarray(o_w[256 * c:256 * c + 256, :]),
            "gate_wt": gate_wt,
            "bias_t": bias_t,
            "sel_t": sel,
            "w_gu": np.ascontiguousarray(gu_f[2 * c:2 * c + 2]),
            "w_dn": np.ascontiguousarray(dn_f[2 * c:2 * c + 2]),
            "cosf": cosf,
            "sinf": sinf,
            "mask_t": mask_t,
            "eye_t": eye_t,
            "reye_t": reye_t,
            "ones_t": ones_t,
        })
    return in_maps


def run(inputs, trace=False, trace_kwargs=None, dbg_outputs=False):
    nc = _get_nc(dbg_outputs)
    in_maps = _make_in_maps(inputs)
    res = run_bass_kernel_spmd(nc, in_maps, list(range(N_CORES)),
                               trace=trace, **(trace_kwargs or {}))
    out_t = np.empty((H, T), np.float32)
    for c in range(N_CORES):
        p = res.results[c]["out_part"]
        for q in range(4):
            out_t[512 * q + 64 * c: 512 * q + 64 * c + 64] = \
                p[64 * q:64 * q + 64]
    out = np.ascontiguousarray(out_t.T).reshape(1, T, H).astype(np.float32)
    return out, res


def kernel(**inputs):
    out, _ = run(inputs, trace=False)
    return out
